# revision 36
# baseline (speedup 1.0000x reference)
"""KitNET anomaly-detection ensemble (25 tiny tied-weight autoencoders) on 8 Trainium2 cores.

Strategy (block-diagonal feature-permuted formulation):
  - Data-parallel over batch: each of the 8 cores processes B/8 = 16384 samples
    in 32 tiles of 512.
  - The feature gather x[:, idx], the transpose to feature-major, and the f32->
    bf16 cast all happen on the host: x ships as 4 chunks of 7/7/7/4 AEs, each
    chunk 112 (64) features + a constant ones-row, zero-padded to 128 DRAM
    rows per chunk so every tile-load DMA is a 128-partition pattern that the
    HWDGE spreads across all 16 SDMA engines (113-row transfers bind to ONE
    engine at ~26 GB/s — 16x slower).
  - Per chunk the encode / decode / group-sum matrices are block diagonal:
    encode+decode are 4 matmuls each per tile (vs dense 400x300), with hb
    folded in via the ones-row and vb via a sigmoid(0)=0.5 constant row
    (coefficient 2*vb). Sigmoids run as [128,2,512] ACT instructions straight
    from PSUM, no bias port needed.
  - An 8-matmul warm-up burst at kernel start opens the PE HAM clock gate
    (2.4 GHz); the shared 3-buffer PSUM pool keeps the loop-carried
    enc(t+1) <- sigmoid(enc23(t)) chain short so it stays open.
  - err = x - rec and err^2 run on the vector engine (bf16 2x, split per half
    so the first half starts as soon as its sigmoid lands); per-AE mean
    squared errors are a 32-wide G matmul (entries 1/16, delayed one tile so
    it never stalls the PE queue) accumulated for 4 batch tiles into one PSUM
    bank via tile_position column offsets.
  - sqrt(S + eps) is phase-split to the end (single ACT table switch, 2
    halves); the final sum over the 25 AEs is 4 concurrent col-group matmuls
    per half against a ones matrix, one bulk PSUM->SBUF copy, 4 output DMAs.
"""

import sys

for _p in ("/opt/trn_rl_repo", "/opt/pypackages"):
    if _p not in sys.path:
        sys.path.append(_p)

import numpy as np

B = 131072
F = 400          # features
N_AE = 25
KF = 16          # features per AE
H = 12           # hidden per AE
EPS = 1e-6
N_CORES = 8
BC = B // N_CORES    # 16384 samples per core
NB = 512             # batch tile (matmul moving free dim)
NT = BC // NB        # 32 tiles per core
NG = NT // 4         # 8 groups of 4 tiles sharing one PSUM S bank

NAE_C = (7, 7, 7, 4)                      # AEs per 128-partition chunk
CR = tuple(1 + KF * n for n in NAE_C)     # contraction rows (1 ones-row + feats)
HR = tuple(1 + H * n for n in NAE_C)      # hidden rows (1 const row + hiddens)

_NC_CACHE = {}


def _build_nc():
    import concourse.tile as tile
    from concourse import bacc, mybir

    f32 = mybir.dt.float32
    bf16 = mybir.dt.bfloat16
    AF = mybir.ActivationFunctionType

    nc = bacc.Bacc()

    x_d = nc.declare_dram_parameter(
        "x", [NT // 2, 2, 4, 128, NB], bf16, isOutput=False
    )
    wenc_d = nc.declare_dram_parameter("wenc", [4, 128, 128], bf16, isOutput=False)
    wdec_d = nc.declare_dram_parameter("wdec", [4, 128, 128], bf16, isOutput=False)
    g_d = nc.declare_dram_parameter("gmat", [4, 128, 32], bf16, isOutput=False)
    ones_d = nc.declare_dram_parameter("ones4", [128, 4], f32, isOutput=False)
    y_d = nc.declare_dram_parameter("y", [BC], f32, isOutput=True)

    with tile.TileContext(nc) as tc:
        with (
            tc.tile_pool(name="singles", bufs=1) as singles,
            tc.tile_pool(name="xt", bufs=3) as xt_p,
            tc.tile_pool(name="ht", bufs=4) as ht_p,
            tc.tile_pool(name="rec", bufs=5) as rec_p,
            tc.tile_pool(name="mmp", bufs=3, space="PSUM") as mmp,
            tc.tile_pool(name="sp", bufs=1, space="PSUM") as sp_p,
            tc.tile_pool(name="yp", bufs=1, space="PSUM") as yp_p,
        ):
            # --- constants ---
            wenc_sb = singles.tile([128, 4, 128], bf16)
            nc.scalar.dma_start(
                out=wenc_sb, in_=wenc_d[:, :, :].rearrange("c p n -> p c n")
            )
            wdec_sb = singles.tile([128, 4, 128], bf16)
            nc.scalar.dma_start(
                out=wdec_sb, in_=wdec_d[:, :, :].rearrange("c p n -> p c n")
            )
            g_sb = singles.tile([128, 4, 32], bf16)
            nc.scalar.dma_start(out=g_sb, in_=g_d[:, :, :].rearrange("c p n -> p c n"))
            ones_sb = singles.tile([128, 4], f32)
            nc.scalar.dma_start(out=ones_sb, in_=ones_d[:, :])
            eps_sb = singles.tile([128, 1], f32)
            nc.gpsimd.memset(eps_sb, EPS)
            # per-(AE, group) mean-squared errors for the whole core:
            # sall[32*(t%4) + 8*c + a', t//4, i]
            sall = singles.tile([128, NG, NB], f32)
            ybuf = singles.tile([128, 2, NB], f32)

            # ---- PE warm-up: ~7 us of back-to-back matmuls so the HAM
            # clock gate opens (K=8/8) before the main loop; the loop's
            # short gaps then never re-throttle it.
            warm = sp_p.tile([128, NB], f32, tag="s")
            for w in range(8):
                nc.tensor.matmul(
                    warm[0:128, 0:340],
                    lhsT=wenc_sb[:, 0, :],
                    rhs=wenc_sb.rearrange("p c n -> p (c n)")[:, 0:340],
                    start=True,
                    stop=True,
                )

            g_state = {}

            def emit_g(r, tg):
                gg = tg % 4
                if gg == 0:
                    S = sp_p.tile([128, NB], f32, tag="s")
                    g_state["S"] = S
                S = g_state["S"]
                for c in range(4):
                    nc.tensor.matmul(
                        S[32 * gg:32 * (gg + 1), :],
                        lhsT=g_sb[:, c, :],
                        rhs=r[:, c, :],
                        start=(c == 0),
                        stop=(c == 3),
                        tile_position=(0, 32 * gg),
                    )
                if gg == 3:
                    nc.vector.tensor_copy(out=sall[:, tg // 4, :], in_=S)

            prev_rec = None
            for tp in range(NT // 2):
                # ---- load 2 tiles = 1024 samples, feature-major bf16
                # (one contiguous 8 KB line per partition in DRAM)
                xt = xt_p.tile([128, 2, 4, NB], bf16, tag="xt")
                for uu in range(2):
                    nc.sync.dma_start(
                        out=xt[:, uu],
                        in_=x_d[tp, uu].rearrange("c p i -> p c i"),
                    )
                for u in range(2):
                    t = 2 * tp + u
                    g = t % 4

                    # ---- encode: ht = sigmoid(Wenc^T @ xt)  (hb via ones-row)
                    ht = ht_p.tile([128, 4, NB], bf16, tag="ht")
                    for half in range(2):
                        pe_ = mmp.tile([128, 2, NB], f32, tag="mm")
                        for cc in range(2):
                            c = 2 * half + cc
                            nc.tensor.matmul(
                                pe_[:, cc, :],
                                lhsT=wenc_sb[:, c, :],
                                rhs=xt[:, u, c, :],
                                start=True,
                                stop=True,
                            )
                        nc.scalar.activation(
                            out=ht[:, 2 * half:2 * half + 2, :],
                            in_=pe_,
                            func=AF.Sigmoid,
                        )

                    # ---- decode: rec = sigmoid(Wdec^T @ ht)  (vb via 0.5-row)
                    rec = rec_p.tile([128, 4, NB], bf16, tag="rec")
                    for half in range(2):
                        pd = mmp.tile([128, 2, NB], f32, tag="mm")
                        for cc in range(2):
                            c = 2 * half + cc
                            nc.tensor.matmul(
                                pd[:, cc, :],
                                lhsT=wdec_sb[:, c, :],
                                rhs=ht[:, c, :],
                                start=True,
                                stop=True,
                            )
                        nc.scalar.activation(
                            out=rec[:, 2 * half:2 * half + 2, :],
                            in_=pd,
                            func=AF.Sigmoid,
                        )

                    # ---- err^2 in place (DVE, bf16 2x), per half so the
                    # first half starts as soon as its sigmoid lands
                    for half in range(2):
                        hs = slice(2 * half, 2 * half + 2)
                        nc.vector.tensor_sub(
                            rec[:, hs], xt[:, u, hs, :], rec[:, hs]
                        )
                        nc.vector.tensor_mul(rec[:, hs], rec[:, hs], rec[:, hs])

                    # ---- per-AE mean for the PREVIOUS tile (one-tile delay so
                    # these matmuls never stall the PE queue on the DVE chain):
                    # S[32g + 8c + a'] += G^T @ err2
                    if prev_rec is not None:
                        emit_g(prev_rec, t - 1)
                    prev_rec = rec

            emit_g(prev_rec, NT - 1)

            # ---- phase B: rmse = sqrt(S + eps); y = sum over AEs
            # Each half: one sqrt over 4 groups, then 4 ysum matmuls into
            # one PSUM bank at 32-row col-group offsets (concurrent in the
            # PE array), one bulk copy out.
            for hh in range(2):
                js = slice(hh * (NG // 2), (hh + 1) * (NG // 2))
                nc.scalar.activation(
                    out=sall[:, js, :], in_=sall[:, js, :], func=AF.Sqrt,
                    bias=eps_sb, scale=1.0,
                )
                py = yp_p.tile([128, NB], f32, tag="y")
                for k in range(NG // 2):
                    j = hh * (NG // 2) + k
                    nc.tensor.matmul(
                        py[32 * k:32 * k + 4, :],
                        lhsT=ones_sb,
                        rhs=sall[:, j, :],
                        start=True,
                        stop=True,
                        tile_position=(0, 32 * k),
                    )
                nc.vector.tensor_copy(out=ybuf[:, hh, :], in_=py)
            # y[b], b = t*NB + i, t = 4j + g, j = 4*hh + k
            # ybuf[32k + g, hh, i]  ->  y view [k][g, hh, i]
            y_ap = y_d[:].rearrange("(hh k g i) -> k g hh i", k=4, g=4, i=NB)
            for k in range(4):
                nc.sync.dma_start(
                    out=y_ap[k], in_=ybuf[32 * k:32 * k + 4, :, :]
                )

    nc.compile()
    return nc


def _host_mats(W, hb, vb, idx):
    import ml_dtypes

    bf16 = ml_dtypes.bfloat16
    W = np.asarray(W, np.float32)
    hb = np.asarray(hb, np.float32)
    vb = np.asarray(vb, np.float32)
    idx = np.asarray(idx)

    wenc = np.zeros((4, 128, 128), np.float32)
    wdec = np.zeros((4, 128, 128), np.float32)
    gmat = np.zeros((4, 128, 32), np.float32)
    ones4 = np.zeros((128, 4), np.float32)
    for c in range(4):
        for ap in range(NAE_C[c]):
            a = 7 * c + ap
            fr = 1 + KF * ap          # feature row base (within chunk)
            hr = 1 + H * ap           # hidden row/col base
            wenc[c, 0, hr:hr + H] = hb[a, :]
            wenc[c, fr:fr + KF, hr:hr + H] = W[a, :, :]
            wdec[c, 0, fr:fr + KF] = 2.0 * vb[a, :]
            wdec[c, hr:hr + H, fr:fr + KF] = W[a, :, :].T
            gmat[c, fr:fr + KF, 8 * c + ap] = 1.0 / KF
            ones4[np.arange(4) * 32 + 8 * c + ap, np.arange(4)] = 1.0

    return {
        "wenc": np.ascontiguousarray(wenc.astype(bf16)),
        "wdec": np.ascontiguousarray(wdec.astype(bf16)),
        "gmat": np.ascontiguousarray(gmat.astype(bf16)),
        "ones4": ones4,
    }


def _host_x(x, idx):
    """Full x [B, 400] f32 -> per-core [NT//2, 113, 2, 4, NB] bf16,
    feature-major, AE-grouped order, ones row at each chunk's row 0.
    DRAM layout gives each 2-tile load one contiguous 8 KB line per
    partition."""
    import ml_dtypes

    bf16 = ml_dtypes.bfloat16
    perm = np.asarray(idx).reshape(-1)          # AE-major feature order
    xg = np.asarray(x, np.float32)[:, perm]     # [B, 400] gather
    xt = xg.T                                   # [400, B] view
    out = []
    for core in range(N_CORES):
        xc = np.zeros((NT // 2, 2, 4, 128, NB), np.float32)
        xc[:, :, :, 0, :] = 1.0
        sl = xt[:, core * BC:(core + 1) * BC]   # [400, BC] view
        slr = sl.reshape(400, NT // 2, 2, NB)   # [f, tp, u, i]
        for c in range(4):
            w = KF * NAE_C[c]
            # [w, tp, u, i] -> [tp, u, w, i]
            xc[:, :, c, 1:1 + w, :] = slr[112 * c:112 * c + w].transpose(
                1, 2, 0, 3
            )
        out.append(xc.astype(bf16))
    return out


def _get_nc():
    if "nc" not in _NC_CACHE:
        _NC_CACHE["nc"] = _build_nc()
    return _NC_CACHE["nc"]


def _run(x, W, hb, vb, idx, trace=False):
    from concourse.bass_utils import run_bass_kernel_spmd

    consts = _host_mats(W, hb, vb, idx)
    xcores = _host_x(x, idx)
    in_maps = [{"x": xcores[c], **consts} for c in range(N_CORES)]
    nc = _get_nc()
    res = run_bass_kernel_spmd(nc, in_maps, list(range(N_CORES)), trace=trace)
    y = np.concatenate([res.results[c]["y"] for c in range(N_CORES)])
    return y, res


def kernel(x, W, hb, vb, idx):
    y, _ = _run(x, W, hb, vb, idx)
    return y
